# revision 10
# baseline (speedup 1.0000x reference)
"""Trainium2 Bass kernel for nn_DifferentiableAggregation_avg (segment reduce).

Strategy (per sharding hint): partition the 262144 output segments across the
8 cores (32768 segments each, disjoint), so no cross-core reduction is needed.
On the host, rows are sorted by segment id and laid out into a per-core padded
layout: each tile of 128 segments (one per SBUF partition) gets a uniform
per-tile slot capacity (max row count over the tile, quantized to 8; tight
because segments are sorted by count). Tiles with equal capacity are grouped
into "super-tiles" so the device works on a few large DMAs and a few large
grouped ops instead of thousands of tiny ones.

The device streams the layout and performs all of the reference's arithmetic:
per-row 3-class max, per-segment sums of logit0, logit1+logit2 and row-max,
label==4 / label==1 counts, and the final sigmoid combine.
"""
import sys

sys.path.insert(0, "/opt/trn_rl_repo")

import numpy as np

NSEG = 262144
NCORES = 8
SEGS_PER_CORE = NSEG // NCORES  # 32768
PART = 128
T = SEGS_PER_CORE // PART  # 256 tiles per core
CAPQ = 8  # capacity quantum
SORTQ = True  # sort segs by (quantized c1, c2) so both caps are tight
MAXSLOTS = 1024  # max G*cap slots per logit super-tile (per partition)
MAXSLOTS_B = 2048  # same for label super-tiles
RAMP = 0  # if >0, limit the first supertiles to G<=RAMP tiles (faster pipeline ramp)
WORKBUFS = 5
SCRBUFS = 3
ACT_S12_T0 = 55  # supertiles starting at tile >= this: s12 via per-tile ACT accum
POOL_ACC_T0 = 1 << 30  # disabled: walrus rejects Pool tensor_scalar with accum


def _split_multiwaits(nc, max_waits=1):
    """walrus codegen in this container only encodes one sync wait on ctrl
    ops (Drain): hoist extra waits onto single-wait no-ops just before."""
    import concourse.mybir as mybir

    n = 0
    for f in nc.m.functions:
        for bb in f.blocks:
            new_insts = []
            for ins in bb.instructions:
                si = getattr(ins, "sync_info", None)
                if si is not None and si.on_wait and len(si.on_wait) > max_waits:
                    waits = list(si.on_wait)
                    for w in waits[:-max_waits]:
                        nop = mybir.InstNoOp(
                            name=f"I-splitwait-{n}",
                            engine=ins.engine,
                            sync_info=mybir.SyncInfo(on_wait=[w], on_update=[]),
                        )
                        n += 1
                        new_insts.append(nop)
                    ins.sync_info = mybir.SyncInfo(
                        on_wait=waits[-max_waits:], on_update=list(si.on_update)
                    )
                new_insts.append(ins)
            bb.instructions = new_insts
    return n


def _supertiles(caps, maxslots=None):
    """Group consecutive tiles with equal cap into (t0, G, cap) chunks."""
    if maxslots is None:
        maxslots = MAXSLOTS
    sts = []
    t = 0
    n = len(caps)
    while t < n:
        cap = int(caps[t])
        g = 1
        gmax = max(1, maxslots // cap)
        if RAMP and t < 16:
            gmax = min(gmax, RAMP)
        while t + g < n and int(caps[t + g]) == cap and g < gmax:
            g += 1
        sts.append((t, g, cap))
        t += g
    return sts


def _tile_maps(sts, ntiles):
    """Per-tile lookup arrays for the scatter formula."""
    stb = np.zeros(ntiles, np.int64)  # base slot offset of tile's super-tile
    sgc = np.zeros(ntiles, np.int64)  # G*cap of its super-tile
    soff = np.zeros(ntiles, np.int64)  # (t-t0)*cap
    base = 0
    for t0, g, cap in sts:
        for i in range(g):
            stb[t0 + i] = base
            sgc[t0 + i] = g * cap
            soff[t0 + i] = i * cap
        base += PART * g * cap
    return stb, sgc, soff, base


def build_nc(cap1, cap2, ntiles, split=True):
    """Per-core Bass program. Same super-tile schedule on all cores. Inputs:
      L  : flat f32 [tot1]   padded logit rows (super-tile-major, partition-major)
      B  : flat f32 [tot2]   padded label rows
      C  : f32 [128, ntiles] true per-segment row counts
    Output:
      out: f32 [128, 2*ntiles]  (j0, j1) per tile column
    """
    import concourse.bass as bass
    import concourse.mybir as mybir
    from concourse.tile import TileContext

    f32 = mybir.dt.float32
    Alu = mybir.AluOpType
    Act = mybir.ActivationFunctionType
    X = mybir.AxisListType.X
    XY = mybir.AxisListType.XY

    st1 = _supertiles(cap1, MAXSLOTS)
    st2 = _supertiles(cap2, MAXSLOTS_B)
    stb1, _, _, tot1s = _tile_maps(st1, ntiles)
    stb2, _, _, tot2s = _tile_maps(st2, ntiles)

    u8 = mybir.dt.uint8
    nc = bass.Bass("TRN2")
    L = nc.dram_tensor("L", [tot1s * 3], f32, kind="ExternalInput")
    B = nc.dram_tensor("B", [tot2s], u8, kind="ExternalInput")
    C = nc.dram_tensor("C", [PART, ntiles], f32, kind="ExternalInput")
    O = nc.dram_tensor("out", [PART, 2 * ntiles], f32, kind="ExternalOutput")

    # merged emission order: L and B super-tiles sorted by starting tile
    merged = [("L", *s) for s in st1] + [("B", *s) for s in st2]
    merged.sort(key=lambda x: (x[1], x[0]))

    with TileContext(nc) as tc:
        with tc.tile_pool(name="acc", bufs=1) as acc, \
             tc.tile_pool(name="work", bufs=WORKBUFS) as work, \
             tc.tile_pool(name="scr", bufs=SCRBUFS) as scrp:
            s0c = acc.tile([PART, ntiles], f32, tag="s0c", name="s0c")
            s12c = acc.tile([PART, ntiles], f32, tag="s12c", name="s12c")
            smaxc = acc.tile([PART, ntiles], f32, tag="smaxc", name="smaxc")
            c4c = acc.tile([PART, ntiles], f32, tag="c4c", name="c4c")
            c1c = acc.tile([PART, ntiles], f32, tag="c1c", name="c1c")
            ctsb = acc.tile([PART, ntiles], f32, tag="ctsb", name="ctsb")
            outsb = acc.tile([PART, 2 * ntiles], f32, tag="outsb", name="outsb")

            nc.sync.dma_start(ctsb, C[:, :])

            for kind, t0, G, cap in merged:
                if kind == "L":
                    a0 = int(stb1[t0]) * 3
                    w = G * cap
                    Lt = work.tile([PART, w * 3], f32, tag="Lt", name=f"Lt{t0}")
                    nc.sync.dma_start(
                        Lt,
                        L[a0 : a0 + PART * w * 3].rearrange("(p x) -> p x", p=PART),
                    )
                    L4 = Lt.rearrange("p (g s c) -> p g s c", g=G, c=3)
                    cs = slice(t0, t0 + G)
                    nc.vector.tensor_reduce(s0c[:, cs], L4[:, :, :, 0], X, Alu.add)
                    if t0 >= ACT_S12_T0:
                        # offload s12 to the (otherwise idle) ACT engine,
                        # one accumulated copy per tile
                        for i in range(G):
                            a12 = scrp.tile(
                                [PART, cap, 2], f32, tag="a12", name=f"a12_{t0}_{i}"
                            )
                            nc.scalar.activation(
                                a12,
                                L4[:, i, :, 1:3],
                                Act.Copy,
                                accum_out=s12c[:, t0 + i : t0 + i + 1],
                            )
                    else:
                        nc.vector.tensor_reduce(
                            s12c[:, cs], L4[:, :, :, 1:3], XY, Alu.add
                        )
                    m01 = scrp.tile([PART, G, cap], f32, tag="m01", name=f"m01_{t0}")
                    nc.vector.tensor_tensor(
                        m01, L4[:, :, :, 0], L4[:, :, :, 1], Alu.max
                    )
                    m012 = scrp.tile([PART, G, cap], f32, tag="m012", name=f"m012_{t0}")
                    nc.vector.tensor_tensor(m012, m01, L4[:, :, :, 2], Alu.max)
                    nc.vector.tensor_reduce(smaxc[:, cs], m012, X, Alu.add)
                else:
                    a0 = int(stb2[t0])
                    w = G * cap
                    Bt = work.tile([PART, w], u8, tag="Bt", name=f"Bt{t0}")
                    nc.sync.dma_start(
                        Bt, B[a0 : a0 + PART * w].rearrange("(p x) -> p x", p=PART)
                    )
                    cs = slice(t0, t0 + G)
                    B3 = Bt.rearrange("p (g s) -> p g s", g=G)
                    if t0 >= POOL_ACC_T0:
                        # per-tile fused eq+sum entirely on GPSIMD
                        for i in range(G):
                            pe4 = scrp.tile([PART, cap], f32, tag="pe4", name=f"pe4_{t0}_{i}")
                            nc.gpsimd.tensor_scalar(
                                pe4, B3[:, i], 4.0, None, Alu.is_equal,
                                op1=Alu.add, accum_out=c4c[:, t0 + i : t0 + i + 1],
                            )
                            pe1 = scrp.tile([PART, cap], f32, tag="pe1", name=f"pe1_{t0}_{i}")
                            nc.gpsimd.tensor_scalar(
                                pe1, B3[:, i], 1.0, None, Alu.is_equal,
                                op1=Alu.add, accum_out=c1c[:, t0 + i : t0 + i + 1],
                            )
                    else:
                        e4 = scrp.tile([PART, G, cap], f32, tag="e4", name=f"e4_{t0}")
                        nc.gpsimd.tensor_scalar(e4, B3, 4.0, None, Alu.is_equal)
                        nc.vector.tensor_reduce(c4c[:, cs], e4, X, Alu.add)
                        e1 = scrp.tile([PART, G, cap], f32, tag="e1", name=f"e1_{t0}")
                        nc.gpsimd.tensor_scalar(e1, B3, 1.0, None, Alu.is_equal)
                        nc.vector.tensor_reduce(c1c[:, cs], e1, X, Alu.add)

            # final combine on [128, ntiles]
            fin = acc
            safe = fin.tile([PART, ntiles], f32, tag="safe", name="safe")
            nc.vector.tensor_scalar_max(safe, ctsb, 1.0)
            inv = fin.tile([PART, ntiles], f32, tag="inv", name="inv")
            nc.vector.reciprocal(inv, safe)
            avg = fin.tile([PART, ntiles], f32, tag="avg", name="avg")
            nc.vector.tensor_tensor(avg, smaxc, inv, Alu.mult)
            small = fin.tile([PART, ntiles], f32, tag="small", name="small")
            nc.vector.tensor_scalar(small, ctsb, 6.0, None, Alu.is_lt)
            c4m = fin.tile([PART, ntiles], f32, tag="c4m", name="c4m")
            nc.vector.tensor_tensor(c4m, c4c, small, Alu.mult)
            c1m = fin.tile([PART, ntiles], f32, tag="c1m", name="c1m")
            nc.vector.tensor_tensor(c1m, c1c, small, Alu.mult)
            u0 = fin.tile([PART, ntiles], f32, tag="u0", name="u0")
            nc.vector.scalar_tensor_tensor(
                u0, c1m, -5.0, avg, op0=Alu.add, op1=Alu.mult
            )
            u1 = fin.tile([PART, ntiles], f32, tag="u1", name="u1")
            nc.vector.scalar_tensor_tensor(
                u1, c4m, -1.0, avg, op0=Alu.add, op1=Alu.mult
            )
            a0t = fin.tile([PART, ntiles], f32, tag="a0t", name="a0t")
            nc.vector.tensor_tensor(a0t, s0c, u0, Alu.add)
            a1t = fin.tile([PART, ntiles], f32, tag="a1t", name="a1t")
            nc.vector.tensor_tensor(a1t, s12c, u1, Alu.add)
            OS = outsb.rearrange("p (t c) -> p t c", c=2)
            nc.scalar.activation(OS[:, :, 0], a0t, Act.Sigmoid, scale=10.0)
            nc.scalar.activation(OS[:, :, 1], a1t, Act.Sigmoid, scale=10.0)
            nc.sync.dma_start(O[:, :], outsb)

    if split:
        _split_multiwaits(nc)
    return nc


def prepare(sub_logits, original_indices, full_sub_labels, full_original_indices):
    """Host-side shard/sort/pad. Returns (in_maps, seg_order, cap1, cap2)."""
    sub_logits = np.ascontiguousarray(np.asarray(sub_logits, dtype=np.float32))
    seg = np.asarray(original_indices).astype(np.int32)
    lab = np.asarray(full_sub_labels).astype(np.uint8)
    fseg = np.asarray(full_original_indices).astype(np.int32)
    n = seg.shape[0]

    c1 = np.bincount(seg, minlength=NSEG).astype(np.int64)
    c2 = np.bincount(fseg, minlength=NSEG).astype(np.int64)

    # per-core segment ordering by (count1, count2)
    seg_order = np.empty(NSEG, np.int32)
    rank = np.empty(NSEG, np.int32)
    for d in range(NCORES):
        sl = slice(d * SEGS_PER_CORE, (d + 1) * SEGS_PER_CORE)
        key1 = (c1[sl] + CAPQ - 1) // CAPQ if SORTQ else c1[sl]
        o = np.lexsort((c2[sl], key1)).astype(np.int32)
        ids = (d * SEGS_PER_CORE + o).astype(np.int32)
        seg_order[sl] = ids
        rank[ids] = np.arange(SEGS_PER_CORE, dtype=np.int32)

    c1o = c1[seg_order].reshape(NCORES, T, PART)
    c2o = c2[seg_order].reshape(NCORES, T, PART)
    cap1 = c1o.max(axis=(0, 2))
    cap2 = c2o.max(axis=(0, 2))
    cap1 = np.maximum((cap1 + CAPQ - 1) // CAPQ * CAPQ, CAPQ).astype(np.int64)
    cap2 = np.maximum((cap2 + CAPQ - 1) // CAPQ * CAPQ, CAPQ).astype(np.int64)

    st1 = _supertiles(cap1, MAXSLOTS)
    st2 = _supertiles(cap2, MAXSLOTS_B)
    stb1, sgc1, soff1, tot1s = _tile_maps(st1, T)
    stb2, sgc2, soff2, tot2s = _tile_maps(st2, T)

    def scatter(values, segv, counts, stb, sgc, soff, tot, width, dtype=np.float32):
        order = np.argsort(segv, kind="stable")
        sseg = segv[order]
        starts = np.concatenate([[0], np.cumsum(counts)]).astype(np.int64)
        k = np.arange(n, dtype=np.int64) - starts[sseg]
        r = rank[sseg].astype(np.int64)
        tt = r >> 7
        p = r & 127
        slot = stb[tt] + p * sgc[tt] + soff[tt] + k
        core = (sseg >> 15).astype(np.int64)
        out = np.zeros((NCORES, tot * width), dtype)
        flat_idx = core * (tot * width) + slot * width
        big = out.reshape(-1)
        vals = values[order]
        if width == 1:
            big[flat_idx] = vals[:, 0]
        else:
            for ch in range(width):
                big[flat_idx + ch] = vals[:, ch]
        return out

    Lpad = scatter(sub_logits, seg, c1, stb1, sgc1, soff1, tot1s, 3)
    Bpad = scatter(lab.reshape(-1, 1), fseg, c2, stb2, sgc2, soff2, tot2s, 1, np.uint8)

    cts = c1o.transpose(0, 2, 1).astype(np.float32)  # [NCORES, 128, T]

    in_maps = [
        {"L": Lpad[d], "B": Bpad[d], "C": np.ascontiguousarray(cts[d])}
        for d in range(NCORES)
    ]
    return in_maps, seg_order, cap1, cap2


def unshard(results, seg_order):
    out = np.empty((NSEG, 2), np.float32)
    for d in range(NCORES):
        o = results[d]["out"]  # [128, 2T]
        j = o.reshape(PART, T, 2).transpose(1, 0, 2).reshape(SEGS_PER_CORE, 2)
        out[seg_order[d * SEGS_PER_CORE : (d + 1) * SEGS_PER_CORE]] = j
    return out


_CACHE = {}


def kernel(sub_logits, original_indices, full_sub_labels, full_original_indices):
    from concourse.bass_utils import run_bass_kernel_spmd

    in_maps, seg_order, cap1, cap2 = prepare(
        sub_logits, original_indices, full_sub_labels, full_original_indices
    )
    key = (tuple(cap1.tolist()), tuple(cap2.tolist()))
    nc = _CACHE.get(key)
    if nc is None:
        nc = build_nc(cap1, cap2, T)
        _CACHE[key] = nc
    res = run_bass_kernel_spmd(nc, in_maps, core_ids=list(range(NCORES)))
    return unshard(res.results, seg_order)
